# revision 15
# baseline (speedup 1.0000x reference)
"""Trainium2 Bass kernel for nn_L2MLoRAqkv (MoE-routed LoRA QKV projection).

Math (per batch b, expert i = idx[b,0]):
    qkv = x @ W.T + bias
    qkv[:, :D]  += (x @ A_q[i]) @ B_q[i] * SCALE
    qkv[:, -D:] += (x @ A_v[i]) @ B_v[i] * SCALE

Strategy: data-parallel over the batch dim (1 batch per NeuronCore, 8 cores).
On the host we gather each batch's expert and fold the rank-8 LoRA update
into the (transposed) projection weight in float64:
    W_eff[b] = W.T; W_eff[:, :D] += A_q[i] @ B_q[i]; W_eff[:, -D:] += A_v[i] @ B_v[i]
so the device kernel is a single dense GEMM per core:
    Y[4096, 3072] = X[4096, 1024] @ W_eff[1024, 3072] + bias

Everything runs in fp16 (PE: 1 cycle/row, same as f32r, at half the DMA
bytes; end-to-end error ~3e-4 vs the 2e-2 gate).  X is pre-transposed on the
host ([D, T], K-major) so both matmul operands load with K on SBUF
partitions via contiguous DMAs.  The output is stored fp16 and upcast on the
host.

Pipeline details:
- ~8 us of fixed NEFF startup happens before any engine slice; a block of
  dummy matmuls on zeroed SBUF ramps the PE p-state (0.65 -> 2.4 GHz takes
  ~3 us of continuous work) while the first real DMAs land.
- The first x chunk and the first W n-slice are DMA'd as [64, .] partition
  halves so the critical tiles spread across more of the 16 DMA queues
  (a [128, 512] fp16 tile is 128 descriptors on ONE queue ~ 6 us).
- Output stores split into two partition halves across both HWDGE rings.
"""

import os
import sys

import numpy as np

for _p in ("/opt/trn_rl_repo",):
    if _p not in sys.path and os.path.isdir(_p):
        sys.path.insert(0, _p)

B = 8          # batches == cores
T = 4096       # tokens per batch
D = 1024       # model dim (contraction K)
N3 = 3072      # qkv output dim
P = 128        # SBUF partitions
NT = 512       # n-tile (one fp32 PSUM bank)
CHUNK = 512    # token chunk streamed per DMA group
KT = D // P        # 8 k-tiles
NN = N3 // NT      # 6 n-tiles
TT = CHUNK // P    # 4 token sub-tiles per chunk
SCALE = 8.0 / 8.0

WARMUP = 14    # dummy matmuls to ramp the PE p-state while DMAs land

_NC_CACHE = {}


def _build(tokens=T):
    import concourse.tile as tile
    from concourse import bacc, mybir

    nchunk = tokens // CHUNK
    f16 = mybir.dt.float16
    f32 = mybir.dt.float32

    nc = bacc.Bacc(
        "TRN2",
        target_bir_lowering=False,
        debug=False,
        enable_asserts=False,
        num_devices=B,
    )
    xt = nc.dram_tensor("xt", [D, tokens], f16, kind="ExternalInput").ap()
    weff = nc.dram_tensor("weff", [D, N3], f16, kind="ExternalInput").ap()
    biasr = nc.dram_tensor("biasr", [P, N3], f16, kind="ExternalInput").ap()
    y = nc.dram_tensor("y", [tokens, N3], f16, kind="ExternalOutput").ap()

    ld_ctr = [0]
    st_ctr = [0]

    with tile.TileContext(nc) as tc:
        with tc.tile_pool(name="const", bufs=1) as const_pool, \
             tc.tile_pool(name="xin", bufs=3) as xin_pool, \
             tc.tile_pool(name="outp", bufs=4) as out_pool, \
             tc.tile_pool(name="ps", bufs=8, space="PSUM") as psum_pool:

            def ld_eng():
                ld_ctr[0] += 1
                return nc.scalar if ld_ctr[0] % 2 else nc.sync

            def st_eng():
                st_ctr[0] += 1
                return nc.scalar if st_ctr[0] % 2 else nc.sync

            # --- PE warmup: ramp the p-state while startup DMAs land. ---
            wz = const_pool.tile([P, NT], f16)
            nc.vector.memset(wz[:], 0.0)
            wps = psum_pool.tile([P, NT], f32, tag="ps", name="ps")
            for _ in range(WARMUP):
                nc.tensor.matmul(
                    wps[:], lhsT=wz[:, 0:P], rhs=wz[:], start=True, stop=True
                )

            def load_chunk(c):
                # X.T chunk: 8 k-tiles of [128, CHUNK] side by side.
                xc = xin_pool.tile([P, KT, CHUNK], f16, tag="xc", name="xc")
                for k in range(KT):
                    ld_eng().dma_start(
                        xc[:, k, :],
                        xt[k * P : (k + 1) * P, c * CHUNK : (c + 1) * CHUNK],
                    )
                return xc

            w_sb = const_pool.tile([P, KT, N3], f16)

            def load_w_slice(n, split):
                for k in range(KT):
                    src = weff[k * P : (k + 1) * P, n * NT : (n + 1) * NT]
                    if split:
                        # Partition halves spread the slice across more DMA
                        # queues (lower latency at startup).
                        h = P // 2
                        ld_eng().dma_start(w_sb[0:h, k, n * NT : (n + 1) * NT], src[0:h, :])
                        ld_eng().dma_start(w_sb[h:P, k, n * NT : (n + 1) * NT], src[h:P, :])
                    else:
                        ld_eng().dma_start(w_sb[:, k, n * NT : (n + 1) * NT], src)

            # Startup DMA issue order mirrors PE consumption order: the head
            # chunk's x is t-granular ([128,128] pieces) so group (t0, n0)
            # unblocks after ~1.5 MB instead of ~3 MB.
            head = 1
            xc0 = xin_pool.tile([P, KT, CHUNK], f16, tag="xc", name="xc")

            def load_x0_t(t):
                for k in range(KT):
                    ld_eng().dma_start(
                        xc0[:, k, t * P : (t + 1) * P],
                        xt[k * P : (k + 1) * P, t * P : (t + 1) * P],
                    )

            load_x0_t(0)
            load_w_slice(0, split=True)
            for t in range(1, TT):
                load_x0_t(t)
            load_w_slice(1, split=True)
            load_w_slice(2, split=False)
            bias_sb = const_pool.tile([P, N3], f16)
            nc.gpsimd.dma_start(bias_sb[:], biasr[:])
            for n in range(3, NN):
                load_w_slice(n, split=False)
            xcs_head = [xc0]

            def drain(ps, c, t, n, ways=2):
                ob = out_pool.tile([P, NT], f16, tag="ob", name="ob")
                nc.vector.tensor_add(ob[:], ps[:], bias_sb[:, n * NT : (n + 1) * NT])
                # Split the store across both HWDGE rings (parallel queues).
                row = c * CHUNK + t * P
                h = P // ways
                for w in range(ways):
                    st_eng().dma_start(
                        y[row + w * h : row + (w + 1) * h, n * NT : (n + 1) * NT],
                        ob[w * h : (w + 1) * h, :],
                    )

            def do_group(xc, c, t, n, ways=2):
                ps = psum_pool.tile([P, NT], f32, tag="ps", name="ps")
                for k in range(KT):
                    nc.tensor.matmul(
                        ps[:],
                        lhsT=xc[:, k, t * P : (t + 1) * P],
                        rhs=w_sb[:, k, n * NT : (n + 1) * NT],
                        start=(k == 0),
                        stop=(k == KT - 1),
                    )
                drain(ps, c, t, n, ways)

            # Prefetch chunk 1 before the head compute: a store dma_start
            # waiting on its drain blocks later loads on the same ring, so
            # loads must precede stores in ring program order.
            nxt = load_chunk(head) if nchunk > head else None

            # Head chunk n-outer, so matmul groups unblock in weff
            # DMA-arrival order and never outrun the loads.
            for n in range(NN):
                for c in range(head):
                    for t in range(TT):
                        do_group(xcs_head[c], c, t, n)

            # Remaining chunks: weff fully resident.
            for c in range(head, nchunk):
                xc = nxt
                nxt = load_chunk(c + 1) if c + 1 < nchunk else None
                last_c = c == nchunk - 1
                for t in range(TT):
                    for n in range(NN):
                        # 4-way store split on the final groups trims the
                        # end-of-kernel store latency.
                        ways = 4 if (last_c and t == TT - 1) else 2
                        do_group(xc, c, t, n, ways)
    nc.compile()
    return nc


def _get_nc(tokens=T):
    if tokens not in _NC_CACHE:
        _NC_CACHE[tokens] = _build(tokens)
    return _NC_CACHE[tokens]


def _prep_in_maps(inputs):
    x = np.asarray(inputs["x"], dtype=np.float32)
    weight = np.asarray(inputs["weight"], dtype=np.float32)
    bias = np.asarray(inputs["bias"], dtype=np.float32)
    aq = np.asarray(inputs["A_q_pool"], dtype=np.float32)
    bq = np.asarray(inputs["B_q_pool"], dtype=np.float32)
    av = np.asarray(inputs["A_v_pool"], dtype=np.float32)
    bv = np.asarray(inputs["B_v_pool"], dtype=np.float32)
    idx = np.asarray(inputs["idx"]).reshape(B, -1)[:, 0].astype(np.int64)

    wt64 = weight.T.astype(np.float64)  # [D, N3]
    biasr = np.ascontiguousarray(np.broadcast_to(bias.astype(np.float16), (P, N3)))
    xts = x.transpose(0, 2, 1)  # [B, D, T] strided view

    in_maps = []
    for b in range(B):
        i = int(idx[b])
        weff = wt64.copy()
        weff[:, :D] += SCALE * (aq[i].astype(np.float64) @ bq[i].astype(np.float64))
        weff[:, N3 - D:] += SCALE * (av[i].astype(np.float64) @ bv[i].astype(np.float64))
        in_maps.append({
            "xt": np.ascontiguousarray(xts[b]).astype(np.float16),
            "weff": weff.astype(np.float16),
            "biasr": biasr,
        })
    return in_maps


def _run(in_maps, trace=False, **kwargs):
    from concourse.bass_utils import run_bass_kernel_spmd

    nc = _get_nc()
    return run_bass_kernel_spmd(
        nc, in_maps, core_ids=list(range(B)), trace=trace, **kwargs
    )


def kernel(**inputs):
    res = _run(_prep_in_maps(inputs), trace=False)
    return np.stack([r["y"].astype(np.float32) for r in res.results], axis=0)


# revision 20
# speedup vs baseline: 1.0079x; 1.0079x over previous
"""Trainium2 Bass kernel for nn_L2MLoRAqkv (MoE-routed LoRA QKV projection).

Math (per batch b, expert i = idx[b,0]):
    qkv = x @ W.T + bias
    qkv[:, :D]  += (x @ A_q[i]) @ B_q[i] * SCALE
    qkv[:, -D:] += (x @ A_v[i]) @ B_v[i] * SCALE

Strategy: data-parallel over the batch dim (1 batch per NeuronCore, 8 cores).
On the host we gather each batch's expert and fold the rank-8 LoRA update
into the (transposed) projection weight in float64:
    W_eff[b] = W.T; W_eff[:, :D] += A_q[i] @ B_q[i]; W_eff[:, -D:] += A_v[i] @ B_v[i]
so the device kernel is a single dense GEMM per core:
    Y[4096, 3072] = X[4096, 1024] @ W_eff[1024, 3072] + bias

Everything runs in fp16 (PE: 1 cycle/row, same as f32r, at half the DMA
bytes; end-to-end error ~3e-4 vs the 2e-2 gate).  X is pre-transposed on the
host ([D, T], K-major) so both matmul operands load with K on SBUF
partitions via contiguous DMAs.  The output is stored fp16 and upcast on the
host.

Pipeline details:
- ~8 us of fixed NEFF startup happens before any engine slice; a block of
  dummy matmuls on zeroed SBUF ramps the PE p-state (0.65 -> 2.4 GHz takes
  ~3 us of continuous work) while the first real DMAs land.
- The first x chunk and the first W n-slice are DMA'd as [64, .] partition
  halves so the critical tiles spread across more of the 16 DMA queues
  (a [128, 512] fp16 tile is 128 descriptors on ONE queue ~ 6 us).
- Output stores split into two partition halves across both HWDGE rings.
"""

import os
import sys

import numpy as np

for _p in ("/opt/trn_rl_repo",):
    if _p not in sys.path and os.path.isdir(_p):
        sys.path.insert(0, _p)

B = 8          # batches == cores
T = 4096       # tokens per batch
D = 1024       # model dim (contraction K)
N3 = 3072      # qkv output dim
P = 128        # SBUF partitions
NT = 512       # n-tile (one fp32 PSUM bank)
CHUNK = 512    # token chunk streamed per DMA group
KT = D // P        # 8 k-tiles
NN = N3 // NT      # 6 n-tiles
TT = CHUNK // P    # 4 token sub-tiles per chunk
SCALE = 8.0 / 8.0

WARMUP = 14    # dummy matmuls to ramp the PE p-state while DMAs land

_NC_CACHE = {}


def _build(tokens=T):
    import concourse.tile as tile
    from concourse import bacc, mybir

    nchunk = tokens // CHUNK
    f16 = mybir.dt.float16
    f32 = mybir.dt.float32

    nc = bacc.Bacc(
        "TRN2",
        target_bir_lowering=False,
        debug=False,
        enable_asserts=False,
        num_devices=B,
    )
    xt = nc.dram_tensor("xt", [D, tokens], f16, kind="ExternalInput").ap()
    weff = nc.dram_tensor("weff", [D, N3], f16, kind="ExternalInput").ap()
    biasr = nc.dram_tensor("biasr", [P, N3], f16, kind="ExternalInput").ap()
    y = nc.dram_tensor("y", [tokens, N3], f16, kind="ExternalOutput").ap()

    ld_ctr = [0]
    st_ctr = [0]

    with tile.TileContext(nc) as tc:
        with tc.tile_pool(name="const", bufs=1) as const_pool, \
             tc.tile_pool(name="xin", bufs=3) as xin_pool, \
             tc.tile_pool(name="outp", bufs=4) as out_pool, \
             tc.tile_pool(name="outh", bufs=TT * NN) as outh_pool, \
             tc.tile_pool(name="ps", bufs=8, space="PSUM") as psum_pool:

            def ld_eng():
                ld_ctr[0] += 1
                return nc.scalar if ld_ctr[0] % 2 else nc.sync

            def st_eng():
                st_ctr[0] += 1
                return nc.scalar if st_ctr[0] % 2 else nc.sync

            # --- PE warmup: ramp the p-state while startup DMAs land. ---
            wz = const_pool.tile([P, NT], f16)
            nc.vector.memset(wz[:], 0.0)
            wps = psum_pool.tile([P, NT], f32, tag="ps", name="ps")
            for _ in range(WARMUP):
                nc.tensor.matmul(
                    wps[:], lhsT=wz[:, 0:P], rhs=wz[:], start=True, stop=True
                )

            def load_chunk(c):
                # X.T chunk: 8 k-tiles of [128, CHUNK] side by side.
                xc = xin_pool.tile([P, KT, CHUNK], f16, tag="xc", name="xc")
                for k in range(KT):
                    ld_eng().dma_start(
                        xc[:, k, :],
                        xt[k * P : (k + 1) * P, c * CHUNK : (c + 1) * CHUNK],
                    )
                return xc

            w_sb = const_pool.tile([P, KT, N3], f16)

            def load_w_slice(n, split):
                for k in range(KT):
                    src = weff[k * P : (k + 1) * P, n * NT : (n + 1) * NT]
                    if split:
                        # Partition halves spread the slice across more DMA
                        # queues (lower latency at startup).
                        h = P // 2
                        ld_eng().dma_start(w_sb[0:h, k, n * NT : (n + 1) * NT], src[0:h, :])
                        ld_eng().dma_start(w_sb[h:P, k, n * NT : (n + 1) * NT], src[h:P, :])
                    else:
                        ld_eng().dma_start(w_sb[:, k, n * NT : (n + 1) * NT], src)

            # Startup DMA issue order mirrors PE consumption order: the head
            # chunk's x is t-granular ([128,128] pieces) so group (t0, n0)
            # unblocks after ~1.5 MB instead of ~3 MB.
            head = 1
            xc0 = xin_pool.tile([P, KT, CHUNK], f16, tag="xc", name="xc")

            def load_x0_t(t):
                for k in range(KT):
                    ld_eng().dma_start(
                        xc0[:, k, t * P : (t + 1) * P],
                        xt[k * P : (k + 1) * P, t * P : (t + 1) * P],
                    )

            load_x0_t(0)
            load_w_slice(0, split=True)
            for t in range(1, TT):
                load_x0_t(t)
            load_w_slice(1, split=True)
            load_w_slice(2, split=False)
            bias_sb = const_pool.tile([P, N3], f16)
            nc.gpsimd.dma_start(bias_sb[:], biasr[:])
            for n in range(3, NN):
                load_w_slice(n, split=False)
            xcs_head = [xc0]

            def store(ob, c, t, n, ways=2):
                # Split the store across both HWDGE rings (parallel queues).
                row = c * CHUNK + t * P
                h = P // ways
                for w in range(ways):
                    st_eng().dma_start(
                        y[row + w * h : row + (w + 1) * h, n * NT : (n + 1) * NT],
                        ob[w * h : (w + 1) * h, :],
                    )

            def drain(ps, c, t, n, ways=2, defer=None):
                pool = outh_pool if defer is not None else out_pool
                tag = "obh" if defer is not None else "ob"
                ob = pool.tile([P, NT], f16, tag=tag, name=tag)
                nc.vector.tensor_add(ob[:], ps[:], bias_sb[:, n * NT : (n + 1) * NT])
                if defer is not None:
                    # Head-chunk stores are deferred: their dma_starts would
                    # sit ahead of later loads in ring program order and eat
                    # HBM bandwidth during the W-load window.
                    defer.append((ob, c, t, n))
                else:
                    store(ob, c, t, n, ways)

            def do_group(xc, c, t, n, ways=2, defer=None):
                ps = psum_pool.tile([P, NT], f32, tag="ps", name="ps")
                for k in range(KT):
                    nc.tensor.matmul(
                        ps[:],
                        lhsT=xc[:, k, t * P : (t + 1) * P],
                        rhs=w_sb[:, k, n * NT : (n + 1) * NT],
                        start=(k == 0),
                        stop=(k == KT - 1),
                    )
                drain(ps, c, t, n, ways, defer)

            # Prefetch chunk 1 before the head compute (t-granular: the PE
            # resumes on chunk 1 as soon as its first t-slices land).  Loads
            # must precede stores in ring program order: a store dma_start
            # waiting on its drain blocks later loads on the same ring.
            xc1 = xin_pool.tile([P, KT, CHUNK], f16, tag="xc", name="xc")
            for t in range(TT):
                for k in range(KT):
                    ld_eng().dma_start(
                        xc1[:, k, t * P : (t + 1) * P],
                        xt[k * P : (k + 1) * P, CHUNK + t * P : CHUNK + (t + 1) * P],
                    )

            # Head chunk n-outer, so matmul groups unblock in weff
            # DMA-arrival order and never outrun the loads.  Stores deferred.
            deferred = []
            for n in range(NN):
                for t in range(TT):
                    do_group(xc0, 0, t, n, defer=deferred)

            # Chunk 2 loads, then the deferred head stores, then chunk 1
            # compute (t-outer so it consumes xc1's t-slices in order).
            nxt = load_chunk(2) if nchunk > 2 else None
            for ob, c, t, n in deferred:
                store(ob, c, t, n)
            for t in range(TT):
                for n in range(NN):
                    do_group(xc1, 1, t, n)

            # Remaining chunks: weff fully resident.
            for c in range(2, nchunk):
                xc = nxt
                nxt = load_chunk(c + 1) if c + 1 < nchunk else None
                last_c = c == nchunk - 1
                for t in range(TT):
                    for n in range(NN):
                        # 4-way store split on the final groups trims the
                        # end-of-kernel store latency.
                        ways = 4 if (last_c and t == TT - 1) else 2
                        do_group(xc, c, t, n, ways)
    nc.compile()
    return nc


def _get_nc(tokens=T):
    if tokens not in _NC_CACHE:
        _NC_CACHE[tokens] = _build(tokens)
    return _NC_CACHE[tokens]


def _prep_in_maps(inputs):
    x = np.asarray(inputs["x"], dtype=np.float32)
    weight = np.asarray(inputs["weight"], dtype=np.float32)
    bias = np.asarray(inputs["bias"], dtype=np.float32)
    aq = np.asarray(inputs["A_q_pool"], dtype=np.float32)
    bq = np.asarray(inputs["B_q_pool"], dtype=np.float32)
    av = np.asarray(inputs["A_v_pool"], dtype=np.float32)
    bv = np.asarray(inputs["B_v_pool"], dtype=np.float32)
    idx = np.asarray(inputs["idx"]).reshape(B, -1)[:, 0].astype(np.int64)

    wt64 = weight.T.astype(np.float64)  # [D, N3]
    biasr = np.ascontiguousarray(np.broadcast_to(bias.astype(np.float16), (P, N3)))
    xts = x.transpose(0, 2, 1)  # [B, D, T] strided view

    in_maps = []
    for b in range(B):
        i = int(idx[b])
        weff = wt64.copy()
        weff[:, :D] += SCALE * (aq[i].astype(np.float64) @ bq[i].astype(np.float64))
        weff[:, N3 - D:] += SCALE * (av[i].astype(np.float64) @ bv[i].astype(np.float64))
        in_maps.append({
            "xt": np.ascontiguousarray(xts[b]).astype(np.float16),
            "weff": weff.astype(np.float16),
            "biasr": biasr,
        })
    return in_maps


def _run(in_maps, trace=False, **kwargs):
    from concourse.bass_utils import run_bass_kernel_spmd

    nc = _get_nc()
    return run_bass_kernel_spmd(
        nc, in_maps, core_ids=list(range(B)), trace=trace, **kwargs
    )


def kernel(**inputs):
    res = _run(_prep_in_maps(inputs), trace=False)
    return np.stack([r["y"].astype(np.float32) for r in res.results], axis=0)


# revision 29
# speedup vs baseline: 1.0224x; 1.0144x over previous
"""Trainium2 Bass kernel for nn_L2MLoRAqkv (MoE-routed LoRA QKV projection).

Math (per batch b, expert i = idx[b,0]):
    qkv = x @ W.T + bias
    qkv[:, :D]  += (x @ A_q[i]) @ B_q[i] * SCALE
    qkv[:, -D:] += (x @ A_v[i]) @ B_v[i] * SCALE

Strategy: data-parallel over the batch dim (1 batch per NeuronCore, 8 cores).
On the host we gather each batch's expert and fold the rank-8 LoRA update
into the (transposed) projection weight in float64:
    W_eff[b] = W.T; W_eff[:, :D] += A_q[i] @ B_q[i]; W_eff[:, -D:] += A_v[i] @ B_v[i]
so the device kernel is a single dense GEMM per core:
    Y[4096, 3072] = X[4096, 1024] @ W_eff[1024, 3072] + bias
X is pre-transposed on the host ([D, T], K-major) so both matmul operands
load with K on SBUF partitions via fast contiguous DMAs.
"""

import os
import sys

import numpy as np

for _p in ("/opt/trn_rl_repo",):
    if _p not in sys.path and os.path.isdir(_p):
        sys.path.insert(0, _p)

B = 8          # batches == cores
T = 4096       # tokens per batch
D = 1024       # model dim (contraction K)
N3 = 3072      # qkv output dim
P = 128        # SBUF partitions
NT = 512       # n-tile (one fp32 PSUM bank)
CHUNK = 512    # token chunk streamed per DMA group
KT = D // P        # 8 k-tiles
NN = N3 // NT      # 6 n-tiles
TT = CHUNK // P    # 4 token sub-tiles per chunk
SCALE = 8.0 / 8.0

MM_DTYPE = "float16"  # 1 cycle/row on PE (same as f32r) at half the DMA bytes

_NC_CACHE = {}


def _build(mm_dtype_name=MM_DTYPE, tokens=T):
    import concourse.tile as tile
    from concourse import bacc, mybir

    nchunk = tokens // CHUNK
    mmdt = getattr(mybir.dt, mm_dtype_name)
    f32 = mybir.dt.float32

    nc = bacc.Bacc(
        "TRN2",
        target_bir_lowering=False,
        debug=False,
        enable_asserts=False,
        num_devices=B,
    )
    f16 = mybir.dt.float16
    xt = nc.dram_tensor("xt", [D, tokens], mmdt, kind="ExternalInput").ap()
    weff = nc.dram_tensor("weff", [D, N3], mmdt, kind="ExternalInput").ap()
    biasr = nc.dram_tensor("biasr", [P, N3], f32, kind="ExternalInput").ap()
    y = nc.dram_tensor("y", [tokens, N3], f16, kind="ExternalOutput").ap()

    with tile.TileContext(nc) as tc:
        with tc.tile_pool(name="const", bufs=1) as const_pool, \
             tc.tile_pool(name="xin", bufs=3) as xin_pool, \
             tc.tile_pool(name="outp", bufs=4) as out_pool, \
             tc.tile_pool(name="ps", bufs=8, space="PSUM") as psum_pool:

            def load_chunk(c, split=False):
                # X.T chunk: 8 k-tiles of [128, CHUNK] side by side.
                xc = xin_pool.tile([P, KT * CHUNK], mmdt, tag="xc", name="xc")
                for k in range(KT):
                    # TRN2 has two HWDGE rings (sync + scalar); split the
                    # startup-critical chunk across both.
                    eng = nc.scalar if (split and k % 2) else nc.sync
                    eng.dma_start(
                        xc[:, k * CHUNK : (k + 1) * CHUNK],
                        xt[k * P : (k + 1) * P, c * CHUNK : (c + 1) * CHUNK],
                    )
                return xc

            # PE warmup: ~36 dummy matmuls on zeroed SBUF ramp the PE
            # p-state (0.65 -> 2.4 GHz needs ~3 us of continuous work)
            # during the DMA startup window, so the first real groups run
            # at full clock.
            wz = const_pool.tile([P, NT], f16)
            nc.vector.memset(wz[:], 0.0)
            wps = psum_pool.tile([P, NT], f32, tag="ps", name="ps")
            for _ in range(36):
                nc.tensor.matmul(
                    wps[:], lhsT=wz[:, 0:P], rhs=wz[:], start=True, stop=True
                )

            # Chunk 0 of X first: the first matmul group needs it, and the
            # DGE rings drain in issue order.  bias goes via SWDGE.
            head = 1
            xcs_head = [load_chunk(0, split=True)]
            bias_sb = const_pool.tile([P, N3], f32)
            nc.gpsimd.dma_start(bias_sb[:], biasr[:])

            # W_eff resident in SBUF as 8 k-slices side by side: [128, 8*3072].
            # DMA n-slice-major so the first matmul groups unblock early,
            # split across both HWDGE rings for bandwidth.
            w_sb = const_pool.tile([P, KT * N3], mmdt)
            for n in range(NN):
                for k in range(KT):
                    eng = nc.scalar if k % 2 else nc.sync
                    dst = w_sb[:, k * N3 + n * NT : k * N3 + (n + 1) * NT]
                    src = weff[k * P : (k + 1) * P, n * NT : (n + 1) * NT]
                    if n == 1:
                        # n1 is the first slice the PE waits on after the
                        # startup set; partition halves halve its per-queue
                        # latency.
                        h = P // 2
                        eng.dma_start(dst[0:h, :], src[0:h, :])
                        eng.dma_start(dst[h:P, :], src[h:P, :])
                    else:
                        eng.dma_start(dst, src)

            store_ctr = [0]

            def drain(ps, c, t, n, ways=1):
                ob = out_pool.tile([P, NT], f16, tag="ob", name="ob")
                nc.vector.tensor_add(ob[:], ps[:], bias_sb[:, n * NT : (n + 1) * NT])
                # Stores alternate between the two HWDGE rings; the final
                # groups split 4 ways to trim the end-of-kernel latency.
                row = c * CHUNK + t * P
                h = P // ways
                for w in range(ways):
                    eng = nc.scalar if store_ctr[0] % 2 else nc.sync
                    store_ctr[0] += 1
                    eng.dma_start(
                        y[row + w * h : row + (w + 1) * h, n * NT : (n + 1) * NT],
                        ob[w * h : (w + 1) * h, :],
                    )

            def do_group(xc, c, t, n, ways=1):
                ps = psum_pool.tile([P, NT], f32, tag="ps", name="ps")
                for k in range(KT):
                    nc.tensor.matmul(
                        ps[:],
                        lhsT=xc[:, k * CHUNK + t * P : k * CHUNK + (t + 1) * P],
                        rhs=w_sb[:, k * N3 + n * NT : k * N3 + (n + 1) * NT],
                        start=(k == 0),
                        stop=(k == KT - 1),
                    )
                drain(ps, c, t, n, ways)

            # Chunks 0+1 as one superchunk, n-outer, so matmul groups unblock
            # in weff DMA-arrival order and never outrun the loads.
            for n in range(NN):
                for c in range(head):
                    for t in range(TT):
                        do_group(xcs_head[c], c, t, n)

            # Remaining chunks: weff fully resident.
            for c in range(head, nchunk):
                xc = load_chunk(c)
                last = c == nchunk - 1
                for t in range(TT):
                    for n in range(NN):
                        ways = 4 if (last and t == TT - 1) else 1
                        do_group(xc, c, t, n, ways)
    nc.compile()
    return nc


def _get_nc(mm_dtype_name=MM_DTYPE, tokens=T):
    key = (mm_dtype_name, tokens)
    if key not in _NC_CACHE:
        _NC_CACHE[key] = _build(mm_dtype_name, tokens)
    return _NC_CACHE[key]


def _prep_in_maps(inputs):
    x = np.asarray(inputs["x"], dtype=np.float32)
    weight = np.asarray(inputs["weight"], dtype=np.float32)
    bias = np.asarray(inputs["bias"], dtype=np.float32)
    aq = np.asarray(inputs["A_q_pool"], dtype=np.float32)
    bq = np.asarray(inputs["B_q_pool"], dtype=np.float32)
    av = np.asarray(inputs["A_v_pool"], dtype=np.float32)
    bv = np.asarray(inputs["B_v_pool"], dtype=np.float32)
    idx = np.asarray(inputs["idx"]).reshape(B, -1)[:, 0].astype(np.int64)

    wt64 = weight.T.astype(np.float64)  # [D, N3]
    biasr = np.ascontiguousarray(np.broadcast_to(bias, (P, N3)))
    xts = x.transpose(0, 2, 1)  # [B, D, T] strided view

    in_maps = []
    for b in range(B):
        i = int(idx[b])
        weff = wt64.copy()
        weff[:, :D] += SCALE * (aq[i].astype(np.float64) @ bq[i].astype(np.float64))
        weff[:, N3 - D:] += SCALE * (av[i].astype(np.float64) @ bv[i].astype(np.float64))
        in_maps.append({
            "xt": np.ascontiguousarray(xts[b]).astype(np.float16),
            "weff": weff.astype(np.float16),
            "biasr": biasr,
        })
    return in_maps


def _run(in_maps, trace=False, **kwargs):
    from concourse.bass_utils import run_bass_kernel_spmd

    nc = _get_nc()
    return run_bass_kernel_spmd(
        nc, in_maps, core_ids=list(range(B)), trace=trace, **kwargs
    )


def kernel(**inputs):
    res = _run(_prep_in_maps(inputs), trace=False)
    return np.stack([r["y"].astype(np.float32) for r in res.results], axis=0)



# revision 32
# speedup vs baseline: 1.0253x; 1.0028x over previous
"""Trainium2 Bass kernel for nn_L2MLoRAqkv (MoE-routed LoRA QKV projection).

Math (per batch b, expert i = idx[b,0]):
    qkv = x @ W.T + bias
    qkv[:, :D]  += (x @ A_q[i]) @ B_q[i] * SCALE
    qkv[:, -D:] += (x @ A_v[i]) @ B_v[i] * SCALE

Strategy: data-parallel over the batch dim (1 batch per NeuronCore, 8 cores).
On the host we gather each batch's expert and fold the rank-8 LoRA update
into the (transposed) projection weight in float64:
    W_eff[b] = W.T; W_eff[:, :D] += A_q[i] @ B_q[i]; W_eff[:, -D:] += A_v[i] @ B_v[i]
so the device kernel is a single dense GEMM per core:
    Y[4096, 3072] = X[4096, 1024] @ W_eff[1024, 3072] + bias
X is pre-transposed on the host ([D, T], K-major) so both matmul operands
load with K on SBUF partitions via fast contiguous DMAs.
"""

import os
import sys

import numpy as np

for _p in ("/opt/trn_rl_repo",):
    if _p not in sys.path and os.path.isdir(_p):
        sys.path.insert(0, _p)

B = 8          # batches == cores
T = 4096       # tokens per batch
D = 1024       # model dim (contraction K)
N3 = 3072      # qkv output dim
P = 128        # SBUF partitions
NT = 512       # n-tile (one fp32 PSUM bank)
CHUNK = 512    # token chunk streamed per DMA group
KT = D // P        # 8 k-tiles
NN = N3 // NT      # 6 n-tiles
TT = CHUNK // P    # 4 token sub-tiles per chunk
SCALE = 8.0 / 8.0

MM_DTYPE = "float16"  # 1 cycle/row on PE (same as f32r) at half the DMA bytes

_NC_CACHE = {}


def _build(mm_dtype_name=MM_DTYPE, tokens=T):
    import concourse.tile as tile
    from concourse import bacc, mybir

    nchunk = tokens // CHUNK
    mmdt = getattr(mybir.dt, mm_dtype_name)
    f32 = mybir.dt.float32

    nc = bacc.Bacc(
        "TRN2",
        target_bir_lowering=False,
        debug=False,
        enable_asserts=False,
        num_devices=B,
    )
    f16 = mybir.dt.float16
    xt = nc.dram_tensor("xt", [D, tokens], mmdt, kind="ExternalInput").ap()
    weff = nc.dram_tensor("weff", [D, N3], mmdt, kind="ExternalInput").ap()
    biasr = nc.dram_tensor("biasr", [P, N3], f16, kind="ExternalInput").ap()
    y = nc.dram_tensor("y", [tokens, N3], f16, kind="ExternalOutput").ap()

    with tile.TileContext(nc) as tc:
        with tc.tile_pool(name="const", bufs=1) as const_pool, \
             tc.tile_pool(name="xin", bufs=3) as xin_pool, \
             tc.tile_pool(name="outp", bufs=4) as out_pool, \
             tc.tile_pool(name="ps", bufs=8, space="PSUM") as psum_pool:

            def load_chunk(c, split=False):
                # X.T chunk: 8 k-tiles of [128, CHUNK] side by side.
                xc = xin_pool.tile([P, KT * CHUNK], mmdt, tag="xc", name="xc")
                for k in range(KT):
                    # TRN2 has two HWDGE rings (sync + scalar); split the
                    # startup-critical chunk across both.
                    eng = nc.scalar if (split and k % 2) else nc.sync
                    eng.dma_start(
                        xc[:, k * CHUNK : (k + 1) * CHUNK],
                        xt[k * P : (k + 1) * P, c * CHUNK : (c + 1) * CHUNK],
                    )
                return xc

            # PE warmup: ~36 dummy matmuls on zeroed SBUF ramp the PE
            # p-state (0.65 -> 2.4 GHz needs ~3 us of continuous work)
            # during the DMA startup window, so the first real groups run
            # at full clock.
            wz = const_pool.tile([P, NT], f16)
            nc.vector.memset(wz[:], 0.0)
            wps = psum_pool.tile([P, NT], f32, tag="ps", name="ps")
            for _ in range(36):
                nc.tensor.matmul(
                    wps[:], lhsT=wz[:, 0:P], rhs=wz[:], start=True, stop=True
                )

            # Chunk 0 of X first: the first matmul group needs it, and the
            # DGE rings drain in issue order.  bias goes via SWDGE.
            head = 1
            xcs_head = [load_chunk(0, split=True)]
            bias_sb = const_pool.tile([P, N3], f16)
            nc.gpsimd.dma_start(bias_sb[:], biasr[:])

            # W_eff resident in SBUF as 8 k-slices side by side: [128, 8*3072].
            # DMA n-slice-major so the first matmul groups unblock early,
            # split across both HWDGE rings for bandwidth.
            w_sb = const_pool.tile([P, KT * N3], mmdt)
            for n in range(NN):
                for k in range(KT):
                    eng = nc.scalar if k % 2 else nc.sync
                    dst = w_sb[:, k * N3 + n * NT : k * N3 + (n + 1) * NT]
                    src = weff[k * P : (k + 1) * P, n * NT : (n + 1) * NT]
                    if n == 1:
                        # n1 is the first slice the PE waits on after the
                        # startup set; partition halves halve its per-queue
                        # latency.
                        h = P // 2
                        eng.dma_start(dst[0:h, :], src[0:h, :])
                        eng.dma_start(dst[h:P, :], src[h:P, :])
                    else:
                        eng.dma_start(dst, src)

            store_ctr = [0]

            def drain(ps, c, t, n, ways=1):
                ob = out_pool.tile([P, NT], f16, tag="ob", name="ob")
                nc.vector.tensor_add(ob[:], ps[:], bias_sb[:, n * NT : (n + 1) * NT])
                # Stores alternate between the two HWDGE rings; the final
                # groups split 4 ways to trim the end-of-kernel latency.
                row = c * CHUNK + t * P
                h = P // ways
                for w in range(ways):
                    eng = nc.scalar if store_ctr[0] % 2 else nc.sync
                    store_ctr[0] += 1
                    eng.dma_start(
                        y[row + w * h : row + (w + 1) * h, n * NT : (n + 1) * NT],
                        ob[w * h : (w + 1) * h, :],
                    )

            def do_group(xc, c, t, n, ways=1):
                ps = psum_pool.tile([P, NT], f32, tag="ps", name="ps")
                for k in range(KT):
                    nc.tensor.matmul(
                        ps[:],
                        lhsT=xc[:, k * CHUNK + t * P : k * CHUNK + (t + 1) * P],
                        rhs=w_sb[:, k * N3 + n * NT : k * N3 + (n + 1) * NT],
                        start=(k == 0),
                        stop=(k == KT - 1),
                    )
                drain(ps, c, t, n, ways)

            # Chunks 0+1 as one superchunk, n-outer, so matmul groups unblock
            # in weff DMA-arrival order and never outrun the loads.
            for n in range(NN):
                for c in range(head):
                    for t in range(TT):
                        do_group(xcs_head[c], c, t, n)

            # Remaining chunks: weff fully resident.
            for c in range(head, nchunk):
                xc = load_chunk(c)
                last = c == nchunk - 1
                for t in range(TT):
                    for n in range(NN):
                        ways = 4 if (last and t == TT - 1) else 1
                        do_group(xc, c, t, n, ways)
    nc.compile()
    return nc


def _get_nc(mm_dtype_name=MM_DTYPE, tokens=T):
    key = (mm_dtype_name, tokens)
    if key not in _NC_CACHE:
        _NC_CACHE[key] = _build(mm_dtype_name, tokens)
    return _NC_CACHE[key]


def _prep_in_maps(inputs):
    x = np.asarray(inputs["x"], dtype=np.float32)
    weight = np.asarray(inputs["weight"], dtype=np.float32)
    bias = np.asarray(inputs["bias"], dtype=np.float32)
    aq = np.asarray(inputs["A_q_pool"], dtype=np.float32)
    bq = np.asarray(inputs["B_q_pool"], dtype=np.float32)
    av = np.asarray(inputs["A_v_pool"], dtype=np.float32)
    bv = np.asarray(inputs["B_v_pool"], dtype=np.float32)
    idx = np.asarray(inputs["idx"]).reshape(B, -1)[:, 0].astype(np.int64)

    wt64 = weight.T.astype(np.float64)  # [D, N3]
    biasr = np.ascontiguousarray(np.broadcast_to(bias.astype(np.float16), (P, N3)))
    xts = x.transpose(0, 2, 1)  # [B, D, T] strided view

    in_maps = []
    for b in range(B):
        i = int(idx[b])
        weff = wt64.copy()
        weff[:, :D] += SCALE * (aq[i].astype(np.float64) @ bq[i].astype(np.float64))
        weff[:, N3 - D:] += SCALE * (av[i].astype(np.float64) @ bv[i].astype(np.float64))
        in_maps.append({
            "xt": np.ascontiguousarray(xts[b]).astype(np.float16),
            "weff": weff.astype(np.float16),
            "biasr": biasr,
        })
    return in_maps


def _run(in_maps, trace=False, **kwargs):
    from concourse.bass_utils import run_bass_kernel_spmd

    nc = _get_nc()
    return run_bass_kernel_spmd(
        nc, in_maps, core_ids=list(range(B)), trace=trace, **kwargs
    )


def kernel(**inputs):
    res = _run(_prep_in_maps(inputs), trace=False)
    return np.stack([r["y"].astype(np.float32) for r in res.results], axis=0)

